# revision 18
# baseline (speedup 1.0000x reference)
"""Trainium2 Bass kernel for nn_CausalSelfAttention_52905407152466.

BitNet-style causal self-attention, distributed over 8 NeuronCores with an
instance-parallel (batch x head-group) sharding that needs NO collective
before attention:

  - core c owns batch b=c//4 and heads {4j..4j+3} with j=c%4.  It receives
    its batch's x (transposed, fp16) and the column slices of Wq/Wk/Wv for
    its heads, computes q,k,v for all 2048 tokens of its batch locally, and
    runs causal attention for its 4 heads.
  - the ternary weight scales (mean|W|) need the full matrices; each core
    reduces a 1/8 row shard of each W and a 128-byte AllGather combines the
    partial sums (fully overlapped with the x pipeline).
  - y reshards to token-sharded via FOUR quarter AllToAlls, one per head,
    each issued as soon as that head's attention finishes so only the last
    quarter's transfer is exposed.  Arriving quarters are transposed to
    channel-major during attention, so the tail is just quant + Wo matmuls.

Numerics: act_quant int8 values split EXACTLY into two fp8e4m3 operands
(a=fp8(v), b=v-a with |b|<=4; both exact), and ternary weights are exact in
fp8, so every projection runs as DoubleRow fp8 matmuls (2x fp16 throughput)
while reproducing the reference int8xternary products exactly (fp32 psum).
Per-token activation-quant scales are folded into the rope tables (q,k),
the exp scale (sw_q*sw_k/sqrt(D)), the v psum copy (sc*sw_v) and the output
copy (scy*sw_o).  Attention runs in fp16 with the ones-column-in-V
normalizer; diagonal score tiles are processed ragged (only the visible
columns are computed/exponentiated) with a single 128-wide affine_select
boundary mask per tile.
"""

import numpy as np

import concourse.bacc as bacc
import concourse.mybir as mybir
import concourse.tile as tile
from concourse.bass_utils import run_bass_kernel_spmd
from concourse.masks import make_identity

F32 = mybir.dt.float32
F16 = mybir.dt.float16
F8 = mybir.dt.float8e4
I8 = mybir.dt.int8
AX = mybir.AxisListType
OP = mybir.AluOpType
ACTF = mybir.ActivationFunctionType
DR = mybir.MatmulPerfMode.DoubleRow

NCORES = 8
B, T, C = 2, 2048, 1024
H, D = 16, 64
HPC = 4                     # heads per core
HD = HPC * D                # 256 projection channels per core
NCT = C // 128              # 8 channel tiles
NCP = NCT // 2              # 4 channel-tile pairs (DoubleRow)
NTT = T // 128              # 16 token tiles per batch
QB = 512                    # query block
NQB = T // QB               # 4
KT = 128                    # key tile
OT = 512                    # owned output tokens per core (256 per batch)
OTT = OT // 128             # 4
CH = 512                    # q/k projection token chunk
NCH = T // CH               # 4
QSZ = 128 * 2 * D           # a2a quarter slot elems: 128p x 2 tiles x 64
ROPE_BASE = 10000.0

_CACHE = {}


def _host_tables():
    pos = np.arange(T, dtype=np.float64)
    inv = 1.0 / (ROPE_BASE ** (np.arange(0, D, 2, dtype=np.float64) / D))
    ang = pos[None, :] * inv[:, None]              # [32, T]
    cos = np.cos(ang).astype(np.float32).astype(np.float16)
    sin = np.sin(ang).astype(np.float32).astype(np.float16)
    t1 = np.concatenate([cos, cos, cos, cos], axis=0)
    t2 = np.concatenate([sin, sin, sin, sin], axis=0)
    return t1.astype(np.float16), t2.astype(np.float16)


def _host_jt():
    i32 = np.eye(32, dtype=np.float16)
    z = np.zeros((32, 32), np.float16)
    j64 = np.block([[z, -i32], [i32, z]])     # J: Jq[0:32] = -q[32:64]; Jq[32:64] = q[0:32]
    jt = np.block([[j64.T, np.zeros((64, 64), np.float16)],
                   [np.zeros((64, 64), np.float16), j64.T]])
    return jt.astype(np.float16)


def build_program():
    nc = bacc.Bacc("TRN2", target_bir_lowering=False, debug=False,
                   num_devices=NCORES)
    io = {}

    def inp(name, shape, dtype=F32):
        io[name] = nc.declare_dram_parameter(name, list(shape), dtype, isOutput=False)
        return io[name]

    def outp(name, shape, dtype=F32):
        io[name] = nc.declare_dram_parameter(name, list(shape), dtype, isOutput=True)
        return io[name]

    inp("xT16", (C, T), F16)
    inp("Wshards", (128, 4, C))
    for n in ("Wq", "Wk", "Wv"):
        inp(n + "Ts", (C, HD))
    inp("WoT", (C, C))
    inp("ropeT1", (128, T), F16)
    inp("ropeT2", (128, T), F16)
    inp("ropeJT", (128, 128), F16)
    outp("out_slice", (OT, C))

    with tile.TileContext(nc) as tc:
        with tc.tile_pool(name="dram", bufs=1, space="DRAM") as dram:
            ag_in = dram.tile([1, 4], F32)
            ag_out = dram.tile([8, 4], F32)
            a2a = [(dram.tile([NCORES, 2 * QSZ], F16, name=f"a2a_in{q}"),
                    dram.tile([NCORES, 2 * QSZ], F16, name=f"a2a_out{q}"))
                   for q in range(2)]
            _build_body(nc, tc, io, ag_in, ag_out, a2a)
    nc.compile()
    return nc


def _build_body(nc, tc, io, ag_in, ag_out, a2a):
    from contextlib import ExitStack
    from itertools import zip_longest
    es = ExitStack()
    const = es.enter_context(tc.tile_pool(name="const", bufs=1))
    sb = es.enter_context(tc.tile_pool(name="sb", bufs=1))
    front = ExitStack()
    fr = front.enter_context(tc.tile_pool(name="fr", bufs=1))
    wl = front.enter_context(tc.tile_pool(name="wl", bufs=1))
    psA = ExitStack()
    ppa = psA.enter_context(tc.tile_pool(name="ppa", bufs=1, space="PSUM"))

    # ---------------- constants -------------------------------------------
    id16 = const.tile([128, 128], F16)
    make_identity(nc, id16[:])
    id8 = const.tile([128, 128], F8)
    make_identity(nc, id8[:])
    jt = const.tile([128, 128], F16)
    ones_col = const.tile([128, 1], F32)
    nc.gpsimd.memset(ones_col[:], 1.0)
    ones_row = const.tile([1, 128], F32)
    nc.gpsimd.memset(ones_row[:], 1.0)
    ones_row16 = const.tile([1, 128], F16)
    nc.gpsimd.memset(ones_row16[:], 1.0)

    # ---------------- P1: x load + per-token absmax -------------------------
    # x DMAs lead the queues; weight-scale shards stream via the DVE queue so
    # the AllGather can fire ~12us in, overlapped with the absmax pass.
    xT = fr.tile([128, NCT, T], F16)
    xTr = io["xT16"].rearrange("(ct p) t -> p ct t", p=128)
    accA = fr.tile([128, T], F16)
    accB = fr.tile([128, T], F16)
    asums = fr.tile([128, 4], F32)
    for w in range(4):
        wsh = fr.tile([128, 1, C], F32, tag="wsh", bufs=4, name=f"wsh_{w}")
        nc.scalar.dma_start(wsh[:], io["Wshards"][:, w:w + 1])
        nc.vector.tensor_reduce(asums[:, w:w + 1], wsh[:, 0], axis=AX.X,
                                op=OP.add, apply_absolute_value=True)
    for ct in range(NCT):
        eng = nc.sync if ct % 2 == 0 else nc.scalar
        eng.dma_start(xT[:, ct], xTr[:, ct])
    part_ps = ppa.tile([1, 4], F32, tag="pp_small")
    nc.tensor.matmul(part_ps[:], ones_col[:], asums[:], start=True, stop=True)
    part_sb = fr.tile([1, 4], F32)
    nc.vector.tensor_copy(part_sb[:], part_ps[:])
    nc.scalar.dma_start(ag_in[:], part_sb[:])
    for ct in range(NCT):
        acc, veng = ((accA, nc.vector), (accB, nc.gpsimd))[ct % 2]
        if ct < 2:
            veng.tensor_scalar(acc[:], xT[:, ct], 0.0, None, op0=OP.abs_max)
        else:
            veng.tensor_tensor(acc[:], acc[:], xT[:, ct], op=OP.abs_max)
    acc = accA
    nc.vector.tensor_tensor(acc[:], accA[:], accB[:], op=OP.abs_max)
    # rope tables / J arrive behind x on the Act queue (needed ~20us in)
    t1 = const.tile([128, T], F16)
    t2 = const.tile([128, T], F16)
    nc.scalar.dma_start(t1[:], io["ropeT1"][:])
    nc.scalar.dma_start(t2[:], io["ropeT2"][:])
    nc.scalar.dma_start(jt[:], io["ropeJT"][:])
    # q/k/v weight slices queue behind x on the sync queue
    wsts = {}
    for wn in ("Wq", "Wk", "Wv"):
        wst = wl.tile([128, NCT, HD], F32, tag="wst", bufs=3, name=f"wst_{wn}")
        nc.sync.dma_start(wst[:], io[wn + "Ts"].rearrange("(ct p) o -> p ct o", p=128))
        wsts[wn] = wst

    # per-token channel max is a partition (C-axis) reduction on gpsimd
    mx_row = fr.tile([1, T], F32)
    nc.gpsimd.tensor_reduce(mx_row[:], acc[:], axis=AX.C, op=OP.max)
    # AllGather of the four weight |.| partial sums (fires ~12us, overlapped)
    nc.gpsimd.collective_compute(
        "AllGather", OP.bypass, replica_groups=[list(range(NCORES))],
        ins=[ag_in.opt()], outs=[ag_out.opt()])
    gath = fr.tile([8, 4], F32)
    nc.scalar.dma_start(gath[:], ag_out[:])

    # ---------------- P1b: per-token scales ---------------------------------
    sc_row = mx_row   # in-place: mx_row becomes the dequant scale row
    nc.vector.tensor_scalar(sc_row[:], mx_row[:], 1e-5, 1.0 / 127.0,
                            op0=OP.max, op1=OP.mult)
    st_row = fr.tile([1, T], F32)
    scrow16 = fr.tile([1, T], F16)
    nc.gpsimd.tensor_copy(scrow16[:], sc_row[:])
    nc.vector.reciprocal(st_row[:], sc_row[:])

    # broadcast sc (f16, dequant) then st (f32, grid-exact) along partitions
    scb_ps = ppa.tile([128, T], F32, tag="bigps")
    nc.tensor.matmul(scb_ps[:], ones_row16[:], scrow16[:], start=True, stop=True)
    scb16 = fr.tile([128, T], F16)
    nc.gpsimd.tensor_copy(scb16[:], scb_ps[:])
    t1s = sb.tile([128, T], F16)
    t2s = sb.tile([128, T], F16)
    nc.vector.tensor_tensor(t1s[:], t1[:], scb16[:], op=OP.mult)
    nc.gpsimd.tensor_tensor(t2s[:], t2[:], scb16[:], op=OP.mult)
    stb = ppa.tile([128, T], F32, tag="bigps", name="stb")  # psum through quant
    nc.tensor.matmul(stb[:], ones_row[:], st_row[:], start=True, stop=True)

    # token-major v-copy scale sv[p, tt] = sc(token 128*tt+p) * sw_v, via
    # transposing the (all-rows-identical) broadcast back per token tile
    svsc = fr.tile([128, NTT, 1], F32)
    for tt in range(NTT):
        trs = ppa.tile([128, 128], F16, tag="trs", bufs=1, name=f"trs_{tt}")
        nc.tensor.transpose(trs[:], scb16[:, 128 * tt:128 * (tt + 1)], id16[:])
        nc.vector.tensor_copy(svsc[:, tt], trs[:, 0:1])
    swc = {}
    inv_s = {}
    tot_ps = ppa.tile([1, 4], F32, tag="pp_small", name="tot_ps")
    nc.tensor.matmul(tot_ps[:], ones_col[0:8], gath[:], start=True, stop=True)
    sw_row = fr.tile([1, 4], F32)
    nc.vector.tensor_scalar(sw_row[:], tot_ps[:], 1.0 / (C * C), 1e-5,
                            op0=OP.mult, op1=OP.max)
    for wi, wn in enumerate(("Wq", "Wk", "Wv", "Wo")):
        swb_ps = ppa.tile([128, 1], F32, tag="pp_small2", name=f"swb_{wn}")
        nc.tensor.matmul(swb_ps[:], ones_row[:], sw_row[:, wi:wi + 1],
                         start=True, stop=True)
        c_ = sb.tile([128, 1], F32, name=f"swc_{wn}")
        nc.vector.tensor_copy(c_[:], swb_ps[:])
        swc[wn] = c_
        iv = sb.tile([128, 1], F32, name=f"invs_{wn}")
        nc.vector.reciprocal(iv[:], c_[:])
        inv_s[wn] = iv
    sv = fr.tile([128, NTT, 1], F32)
    nc.vector.tensor_scalar(sv[:], svsc[:], swc["Wv"][:], None, op0=OP.mult)

    # quantize: xq8 int8 grid == fp8 a + fp8 b exactly
    a8 = fr.tile([128, NCT, T], F8)
    b8 = fr.tile([128, NCT, T], F8)
    for ct in range(NCT):
        xq8 = fr.tile([128, T], I8, tag="xq8", bufs=2, name=f"xq8_{ct}")
        nc.vector.tensor_tensor(xq8[:], xT[:, ct], stb[:], op=OP.mult)
        nc.scalar.activation(a8[:, ct], xq8[:], ACTF.Copy)
        beng = nc.gpsimd if ct % 2 == 0 else nc.vector
        beng.tensor_tensor(b8[:, ct], xq8[:], a8[:, ct], op=OP.subtract)

    # ---------------- P2: ternarize weight slices to fp8 --------------------
    w8 = {}
    for i, wn in enumerate(("Wq", "Wk", "Wv")):
        wi8 = fr.tile([128, NCT, HD], I8, tag="wi8", bufs=2, name=f"wi8_{wn}")
        e1, e2 = (nc.vector, nc.gpsimd) if i % 2 == 0 else (nc.gpsimd, nc.vector)
        e1.tensor_scalar(wi8[:], wsts[wn][:], inv_s[wn][:], None, op0=OP.mult)
        wf = sb.tile([128, NCT, HD], F8, name=f"w8_{wn}")
        e2.tensor_scalar(wf[:], wi8[:], 1, -1, op0=OP.min, op1=OP.max)
        w8[wn] = wf

    # ---------------- P3: projections --------------------------------------
    psA.close()
    psB = ExitStack()
    ppb = psB.enter_context(tc.tile_pool(name="ppb", bufs=1, space="PSUM"))

    q_sb = sb.tile([128, 2, T], F16)
    k_sb = sb.tile([128, 2, T], F16)
    for name, dst in (("Wq", q_sb), ("Wk", k_sb)):
        for p in range(2):
            for ch in range(NCH):
                tsl = slice(CH * ch, CH * (ch + 1))
                mm = ppb.tile([128, CH], F32, tag="mmq", bufs=2,
                              name=f"mm_{name}{p}{ch}")
                for ctp in range(NCP):
                    for si, src in enumerate((a8, b8)):
                        nc.tensor.matmul(
                            mm[:], w8[name][:, 2 * ctp:2 * ctp + 2, 128 * p:128 * (p + 1)],
                            src[:, 2 * ctp:2 * ctp + 2, tsl],
                            start=(ctp == 0 and si == 0),
                            stop=(ctp == NCP - 1 and si == 1), perf_mode=DR)
                raw = fr.tile([128, CH], F16, tag="raw", bufs=2,
                              name=f"raw_{name}{p}{ch}")
                nc.scalar.activation(raw[:], mm[:], ACTF.Copy)
                jq = ppb.tile([128, CH], F32, tag="jq", bufs=2,
                              name=f"jq_{name}{p}{ch}")
                nc.tensor.matmul(jq[:], jt[:], raw[:], start=True, stop=True)
                p1 = fr.tile([128, CH], F16, tag="p1", bufs=2,
                             name=f"p1_{name}{p}{ch}")
                nc.vector.tensor_tensor(p1[:], mm[:], t1s[:, tsl], op=OP.mult)
                p2t = fr.tile([128, CH], F16, tag="p2", bufs=2,
                              name=f"p2_{name}{p}{ch}")
                nc.gpsimd.tensor_tensor(p2t[:], jq[:], t2s[:, tsl], op=OP.mult)
                nc.vector.tensor_tensor(dst[:, p, tsl], p1[:], p2t[:], op=OP.add)

    v_sb = sb.tile([128, NTT, HPC, 65], F16)
    nc.gpsimd.memset(v_sb[:, :, :, 64:65], 1.0)
    for tt in range(NTT):
        mmv = ppb.tile([128, HD], F32, tag="mmv", bufs=3, name=f"mmv_{tt}")
        for ctp in range(NCP):
            for si, src in enumerate((a8, b8)):
                nc.tensor.matmul(
                    mmv[:], src[:, 2 * ctp:2 * ctp + 2, 128 * tt:128 * (tt + 1)],
                    w8["Wv"][:, 2 * ctp:2 * ctp + 2, :],
                    start=(ctp == 0 and si == 0),
                    stop=(ctp == NCP - 1 and si == 1), perf_mode=DR)
        nc.scalar.activation(v_sb[:, tt, :, 0:64],
                             mmv[:].rearrange("p (h d) -> p h d", h=HPC),
                             ACTF.Copy, scale=sv[:, tt])

    # ---------------- P4: attention + per-pair resharding -------------------
    expsc = sb.tile([128, 1], F32)
    nc.vector.tensor_tensor(expsc[:], swc["Wq"][:], swc["Wk"][:], op=OP.mult)
    nc.vector.tensor_scalar(expsc[:], expsc[:], 1.0 / np.sqrt(np.float64(D)), None,
                            op0=OP.mult)
    swco = sb.tile([128, 1], F32)
    nc.vector.tensor_copy(swco[:], swc["Wo"][:])
    osiv = sb.tile([128, 1], F32)
    nc.vector.tensor_copy(osiv[:], inv_s["Wo"][:])

    front.close()
    psB.close()
    expp = es.enter_context(tc.tile_pool(name="expp", bufs=1))
    tail = es.enter_context(tc.tile_pool(name="tail", bufs=1))
    psC = ExitStack()
    ppc = psC.enter_context(tc.tile_pool(name="ppc", bufs=1, space="PSUM"))

    y_sb = sb.tile([128, NTT, HPC, D], F16)
    # channel-major resharded y (built during attention from half arrivals)
    ycT = tail.tile([128, NCT, OT], F16)
    accY = tail.tile([128, OT], F16)
    nc.gpsimd.memset(accY[:], 0.0)

    # Wo loads + ternarize overlap attention (DMA and DVE are mostly idle)
    wo8 = tail.tile([128, NCT, C], F8)
    woR = io["WoT"].rearrange("(ct p) o -> p ct o", p=128)
    for chunk in range(4):
        wst = tail.tile([128, 2, C], F32, tag="woc", bufs=2, name=f"woc_{chunk}")
        nc.sync.dma_start(wst[:], woR[:, 2 * chunk:2 * chunk + 2])
        wi8o = tail.tile([128, 2, C], I8, tag="woi", bufs=2, name=f"woi_{chunk}")
        nc.vector.tensor_scalar(wi8o[:], wst[:], osiv[:], None, op0=OP.mult)
        nc.vector.tensor_scalar(wo8[:, 2 * chunk:2 * chunk + 2], wi8o[:], 1, -1,
                                op0=OP.min, op1=OP.max)

    def head_thunks(p, e):
        """List of emission thunks for head (p,e): one per score group plus
        one per epilogue block; interleaving two heads' thunks keeps the
        exp pipe and PE continuously fed."""
        h = 2 * p + e
        thunks = []
        state = {"first": True, "kt": 0, "yaug": None}

        def mk_yaug(jb):
            def f():
                state["yaug"] = ppc.tile([65, QB], F32, tag=f"yaug{e}",
                                         name=f"yaug{h}{jb}")
                state["first"] = True
            return f

        def av(egrp, esl, csl, stop):
            nc.tensor.matmul(state["yaug"][:, csl], v_sb[:, state["kt"], h, :],
                             egrp[:, esl], start=state["first"], stop=stop,
                             skip_group_check=True)
            state["first"] = False

        def mk_full(jb, fg):
            def f():
                sgrp = ppc.tile([128, 2 * QB], F32, tag="sgrp", bufs=2,
                                name=f"sgrp{h}{jb}{fg}")
                for m in range(2):
                    kt = fg * 2 + m
                    nc.tensor.matmul(
                        sgrp[:, QB * m:QB * (m + 1)],
                        k_sb[64 * e:64 * (e + 1), p, 128 * kt:128 * (kt + 1)],
                        q_sb[64 * e:64 * (e + 1), p, QB * jb:QB * (jb + 1)],
                        start=True, stop=True, tile_position=(64 * e, 0))
                egrp = expp.tile([128, 2 * QB], F16, tag=f"egrp{e}", bufs=3,
                                 name=f"egrp{h}{jb}{fg}")
                nc.scalar.activation(egrp[:], sgrp[:], ACTF.Exp, scale=expsc[:])
                for m in range(2):
                    state["kt"] = fg * 2 + m
                    av(egrp, slice(QB * m, QB * (m + 1)), slice(0, QB),
                       stop=False)
            return f

        def mk_diag(jb, dpair):
            def f():
                widths = [QB - KT * (2 * dpair), QB - KT * (2 * dpair + 1)]
                wtot = sum(widths)
                offs = [0, widths[0]]
                sgrp = ppc.tile([128, wtot], F32, tag="sgrp", bufs=2,
                                name=f"sgrpd{h}{jb}{dpair}")
                for ii in range(2):
                    i = 2 * dpair + ii
                    kt = 4 * jb + i
                    nc.tensor.matmul(
                        sgrp[:, offs[ii]:offs[ii] + widths[ii]],
                        k_sb[64 * e:64 * (e + 1), p, 128 * kt:128 * (kt + 1)],
                        q_sb[64 * e:64 * (e + 1), p,
                             QB * jb + KT * i:QB * (jb + 1)],
                        start=True, stop=True, tile_position=(64 * e, 0))
                egrp = expp.tile([128, wtot], F16, tag=f"egrp{e}", bufs=3,
                                 name=f"egrpd{h}{jb}{dpair}")
                nc.scalar.activation(egrp[:], sgrp[:], ACTF.Exp, scale=expsc[:])
                for ii in range(2):
                    # staircase mask on the first 128 columns of each slice
                    nc.gpsimd.affine_select(
                        out=egrp[:, offs[ii]:offs[ii] + KT],
                        in_=egrp[:, offs[ii]:offs[ii] + KT],
                        compare_op=OP.is_ge, fill=0.0,
                        base=0, pattern=[[1, KT]], channel_multiplier=-1)
                for ii in range(2):
                    i = 2 * dpair + ii
                    state["kt"] = 4 * jb + i
                    av(egrp, slice(offs[ii], offs[ii] + widths[ii]),
                       slice(KT * i, QB),
                       stop=(dpair == 1 and ii == 1))
            return f

        def mk_epi(jb):
            def f():
                yaug16 = expp.tile([65, QB], F16, tag=f"yaug16_{e}", bufs=2,
                                   name=f"yaug16{h}{jb}")
                nc.vector.tensor_copy(yaug16[:], state["yaug"][:])
                for chn in range(QB // 128):
                    trp = ppc.tile([128, 512], F16, tag="trx", bufs=2,
                                   name=f"trp{h}{jb}{chn}")
                    nc.tensor.transpose(trp[:, 0:65],
                                        yaug16[:, 128 * chn:128 * (chn + 1)],
                                        id16[0:65, 0:65])
                    rec = expp.tile([128, 1], F32, tag=f"rec{e}", bufs=2,
                                    name=f"rec{h}{jb}{chn}")
                    nc.vector.reciprocal(rec[:], trp[:, 64:65])
                    nc.vector.tensor_scalar(
                        y_sb[:, 4 * jb + chn, h, :], trp[:, 0:64],
                        rec[:], None, op0=OP.mult)
            return f

        for jb in range(NQB):
            thunks.append(mk_yaug(jb))
            for fg in range(2 * jb):
                thunks.append(mk_full(jb, fg))
            for dpair in range(2):
                thunks.append(mk_diag(jb, dpair))
            thunks.append(mk_epi(jb))
        return thunks

    def attention_pair(p):
        for a, b in zip_longest(head_thunks(p, 0), head_thunks(p, 1)):
            if a is not None:
                a()
            if b is not None:
                b()

    def send_half(ph):
        a2a_in, a2a_out = a2a[ph]
        for d in range(NCORES):
            nc.sync.dma_start(
                a2a_in[d].rearrange("(p t h dd) -> p t h dd", p=128, t=2, h=2),
                y_sb[:, 2 * d:2 * d + 2, 2 * ph:2 * ph + 2, :])
        nc.gpsimd.collective_compute(
            "AllToAll", OP.bypass, replica_groups=[list(range(NCORES))],
            ins=[a2a_in.opt()], outs=[a2a_out.opt()])

    def recv_half(ph):
        """Unpack half ph into channel-major ycT and fold into accY.

        Source s holds global heads {4*(s%4)+2*ph, +1}, i.e. channels
        [256*(s%4)+128*ph, +128) -> ct = 2*(s%4) + ph, lanes 0..127.
        """
        _, a2a_out = a2a[ph]
        for s in range(NCORES):
            ct = 2 * (s % 4) + ph
            yarr = tail.tile([128, 2, 2, D], F16, tag="yarr", bufs=4,
                             name=f"yarr{ph}{s}")
            nc.sync.dma_start(
                yarr[:],
                a2a_out[s].rearrange("(p t h dd) -> p t h dd", p=128, t=2, h=2))
            try8 = ppc.tile([128, 512], F16, tag="trx", bufs=2,
                            name=f"try{ph}{s}")
            for tl in range(2):
                for hh in range(2):
                    nc.tensor.transpose(
                        try8[64 * hh:64 * (hh + 1), 128 * tl:128 * (tl + 1)],
                        yarr[:, tl, hh], id16[:])
            tt_loc = 2 * (s // 4)
            csl = slice(128 * tt_loc, 128 * (tt_loc + 2))
            eng = (nc.vector, nc.gpsimd)[s % 2]
            eng.tensor_copy(ycT[:, ct, csl], try8[:, 0:256])
            eng.tensor_tensor(accY[:, csl], accY[:, csl], try8[:, 0:256],
                              op=OP.abs_max)

    attention_pair(0)
    send_half(0)
    attention_pair(1)
    recv_half(0)
    send_half(1)
    recv_half(1)

    # ---------------- P5: output quant + projection -------------------------
    psC.close()
    psD = ExitStack()
    ppd = psD.enter_context(tc.tile_pool(name="ppd", bufs=1, space="PSUM"))

    mxy_row = tail.tile([1, OT], F32)
    nc.gpsimd.tensor_reduce(mxy_row[:], accY[:], axis=AX.C, op=OP.max)
    scy_row = mxy_row
    nc.vector.tensor_scalar(scy_row[:], mxy_row[:], 1e-5, 1.0 / 127.0,
                            op0=OP.max, op1=OP.mult)
    sty_row = tail.tile([1, OT], F32)
    nc.vector.reciprocal(sty_row[:], scy_row[:])
    scyrow16 = tail.tile([1, OT], F16)
    nc.vector.tensor_copy(scyrow16[:], scy_row[:])
    styb = ppd.tile([128, OT], F32, tag="styb")
    nc.tensor.matmul(styb[:], ones_row[:], sty_row[:], start=True, stop=True)
    scyb_ps = ppd.tile([128, OT], F32, tag="scyb")
    nc.tensor.matmul(scyb_ps[:], ones_row16[:], scyrow16[:], start=True, stop=True)
    scyb16 = tail.tile([128, OT], F16)
    nc.gpsimd.tensor_copy(scyb16[:], scyb_ps[:])
    osc = tail.tile([128, OTT, 1], F32)
    for tt in range(OTT):
        trso = ppd.tile([128, 128], F16, tag="trso", bufs=1, name=f"trso_{tt}")
        nc.tensor.transpose(trso[:], scyb16[:, 128 * tt:128 * (tt + 1)], id16[:])
        nc.vector.tensor_scalar(osc[:, tt], trso[:, 0:1], swco[:], None,
                                op0=OP.mult)

    ya8 = tail.tile([128, NCT, OT], F8)
    yb8 = tail.tile([128, NCT, OT], F8)
    for ct in range(NCT):
        yq8 = tail.tile([128, OT], I8, tag="yq8", bufs=2, name=f"yq8_{ct}")
        nc.vector.tensor_tensor(yq8[:], ycT[:, ct], styb[:], op=OP.mult)
        nc.scalar.activation(ya8[:, ct], yq8[:], ACTF.Copy)
        nc.gpsimd.tensor_tensor(yb8[:, ct], yq8[:], ya8[:, ct], op=OP.subtract)

    outR = io["out_slice"].rearrange("(tt p) c -> p tt c", p=128)
    for tt in range(OTT):
        pso = ppd.tile([128, C], F32, tag="pso", bufs=2, name=f"pso_{tt}")
        for ctp in range(NCP):
            for si, srcT in enumerate((ya8, yb8)):
                nc.tensor.matmul(
                    pso[:], srcT[:, 2 * ctp:2 * ctp + 2, 128 * tt:128 * (tt + 1)],
                    wo8[:, 2 * ctp:2 * ctp + 2, :],
                    start=(ctp == 0 and si == 0),
                    stop=(ctp == NCP - 1 and si == 1), perf_mode=DR)
        outt = tail.tile([128, C], F32, tag="outt", bufs=2, name=f"outt_{tt}")
        nc.scalar.activation(outt[:], pso[:], ACTF.Copy, scale=osc[:, tt])
        nc.sync.dma_start(outR[:, tt], outt[:])
    psD.close()
    es.close()


def kernel(x, Wq, Wk, Wv, Wo, _trace=False):
    x = np.ascontiguousarray(np.asarray(x, np.float32))
    if "nc" not in _CACHE:
        _CACHE["nc"] = build_program()
    nc = _CACHE["nc"]
    t1, t2 = _host_tables()
    jth = _host_jt()
    ws = {"Wq": np.asarray(Wq, np.float32), "Wk": np.asarray(Wk, np.float32),
          "Wv": np.asarray(Wv, np.float32), "Wo": np.asarray(Wo, np.float32)}
    wstack = np.stack([ws["Wq"], ws["Wk"], ws["Wv"], ws["Wo"]], axis=0)
    woT = np.ascontiguousarray(ws["Wo"].T)
    in_maps = []
    for c in range(NCORES):
        b, j = c // 4, c % 4
        in_maps.append({
            "xT16": np.ascontiguousarray(x[b].T.astype(np.float16)),
            "Wshards": np.ascontiguousarray(
                wstack[:, 128 * c:128 * (c + 1), :].transpose(1, 0, 2)),
            "WqTs": np.ascontiguousarray(ws["Wq"][HD * j:HD * (j + 1), :].T),
            "WkTs": np.ascontiguousarray(ws["Wk"][HD * j:HD * (j + 1), :].T),
            "WvTs": np.ascontiguousarray(ws["Wv"][HD * j:HD * (j + 1), :].T),
            "WoT": woT,
            "ropeT1": t1, "ropeT2": t2, "ropeJT": jth,
        })
    res = run_bass_kernel_spmd(nc, in_maps, list(range(NCORES)), trace=_trace)
    out = np.zeros((B, T, C), np.float32)
    for c in range(NCORES):
        o = np.asarray(res.results[c]["out_slice"])
        out[0, 256 * c:256 * (c + 1)] = o[0:256]
        out[1, 256 * c:256 * (c + 1)] = o[256:512]
    if _trace:
        return out, res
    return out


# revision 19
# speedup vs baseline: 1.0389x; 1.0389x over previous
"""Trainium2 Bass kernel for nn_CausalSelfAttention_52905407152466.

BitNet-style causal self-attention, distributed over 8 NeuronCores with an
instance-parallel (batch x head-group) sharding that needs NO collective
before attention:

  - core c owns batch b=c//4 and heads {4j..4j+3} with j=c%4.  It receives
    its batch's x (transposed, fp16) and the column slices of Wq/Wk/Wv for
    its heads, computes q,k,v for all 2048 tokens of its batch locally, and
    runs causal attention for its 4 heads.
  - the ternary weight scales (mean|W|) need the full matrices; each core
    reduces a 1/8 row shard of each W and a 128-byte AllGather combines the
    partial sums (fully overlapped with the x pipeline).
  - y reshards to token-sharded via FOUR quarter AllToAlls, one per head,
    each issued as soon as that head's attention finishes so only the last
    quarter's transfer is exposed.  Arriving quarters are transposed to
    channel-major during attention, so the tail is just quant + Wo matmuls.

Numerics: act_quant int8 values split EXACTLY into two fp8e4m3 operands
(a=fp8(v), b=v-a with |b|<=4; both exact), and ternary weights are exact in
fp8, so every projection runs as DoubleRow fp8 matmuls (2x fp16 throughput)
while reproducing the reference int8xternary products exactly (fp32 psum).
Per-token activation-quant scales are folded into the rope tables (q,k),
the exp scale (sw_q*sw_k/sqrt(D)), the v psum copy (sc*sw_v) and the output
copy (scy*sw_o).  Attention runs in fp16 with the ones-column-in-V
normalizer; diagonal score tiles are processed ragged (only the visible
columns are computed/exponentiated) with a single 128-wide affine_select
boundary mask per tile.
"""

import numpy as np

import concourse.bacc as bacc
import concourse.mybir as mybir
import concourse.tile as tile
from concourse import bass_isa
from concourse.bass_utils import run_bass_kernel_spmd
from concourse.masks import make_identity

F32 = mybir.dt.float32
F16 = mybir.dt.float16
F8 = mybir.dt.float8e4
I8 = mybir.dt.int8
AX = mybir.AxisListType
OP = mybir.AluOpType
ACTF = mybir.ActivationFunctionType
DR = mybir.MatmulPerfMode.DoubleRow

NCORES = 8
B, T, C = 2, 2048, 1024
H, D = 16, 64
HPC = 4                     # heads per core
HD = HPC * D                # 256 projection channels per core
NCT = C // 128              # 8 channel tiles
NCP = NCT // 2              # 4 channel-tile pairs (DoubleRow)
NTT = T // 128              # 16 token tiles per batch
QB = 512                    # query block
NQB = T // QB               # 4
KT = 128                    # key tile
OT = 512                    # owned output tokens per core (256 per batch)
OTT = OT // 128             # 4
CH = 512                    # q/k projection token chunk
NCH = T // CH               # 4
QSZ = 128 * 2 * D           # a2a quarter slot elems: 128p x 2 tiles x 64
ROPE_BASE = 10000.0

_CACHE = {}


def _host_tables():
    pos = np.arange(T, dtype=np.float64)
    inv = 1.0 / (ROPE_BASE ** (np.arange(0, D, 2, dtype=np.float64) / D))
    ang = pos[None, :] * inv[:, None]              # [32, T]
    cos = np.cos(ang).astype(np.float32).astype(np.float16)
    sin = np.sin(ang).astype(np.float32).astype(np.float16)
    t1 = np.concatenate([cos, cos, cos, cos], axis=0)
    t2 = np.concatenate([sin, sin, sin, sin], axis=0)
    return t1.astype(np.float16), t2.astype(np.float16)


def _host_jt():
    i32 = np.eye(32, dtype=np.float16)
    z = np.zeros((32, 32), np.float16)
    j64 = np.block([[z, -i32], [i32, z]])     # J: Jq[0:32] = -q[32:64]; Jq[32:64] = q[0:32]
    jt = np.block([[j64.T, np.zeros((64, 64), np.float16)],
                   [np.zeros((64, 64), np.float16), j64.T]])
    return jt.astype(np.float16)


def build_program():
    nc = bacc.Bacc("TRN2", target_bir_lowering=False, debug=False,
                   num_devices=NCORES)
    io = {}

    def inp(name, shape, dtype=F32):
        io[name] = nc.declare_dram_parameter(name, list(shape), dtype, isOutput=False)
        return io[name]

    def outp(name, shape, dtype=F32):
        io[name] = nc.declare_dram_parameter(name, list(shape), dtype, isOutput=True)
        return io[name]

    inp("xT16", (C, T), F16)
    inp("Wshards", (128, 4, C))
    for n in ("Wq", "Wk", "Wv"):
        inp(n + "Ts", (C, HD))
    inp("WoT", (C, C))
    inp("ropeT1", (128, T), F16)
    inp("ropeT2", (128, T), F16)
    inp("ropeJT", (128, 128), F16)
    outp("out_slice", (OT, C))

    with tile.TileContext(nc) as tc:
        with tc.tile_pool(name="dram", bufs=1, space="DRAM") as dram:
            ag_in = dram.tile([1, 4], F32)
            ag_out = dram.tile([8, 4], F32)
            a2a = [(dram.tile([NCORES, 2 * QSZ], F16, name=f"a2a_in{q}"),
                    dram.tile([NCORES, 2 * QSZ], F16, name=f"a2a_out{q}"))
                   for q in range(2)]
            _build_body(nc, tc, io, ag_in, ag_out, a2a)
    nc.compile()
    return nc


def _build_body(nc, tc, io, ag_in, ag_out, a2a):
    from contextlib import ExitStack
    from itertools import zip_longest
    es = ExitStack()
    const = es.enter_context(tc.tile_pool(name="const", bufs=1))
    sb = es.enter_context(tc.tile_pool(name="sb", bufs=1))
    front = ExitStack()
    fr = front.enter_context(tc.tile_pool(name="fr", bufs=1))
    wl = front.enter_context(tc.tile_pool(name="wl", bufs=1))
    psA = ExitStack()
    ppa = psA.enter_context(tc.tile_pool(name="ppa", bufs=1, space="PSUM"))

    # ---------------- constants -------------------------------------------
    id16 = const.tile([128, 128], F16)
    make_identity(nc, id16[:])
    id8 = const.tile([128, 128], F8)
    make_identity(nc, id8[:])
    jt = const.tile([128, 128], F16)
    ones_col = const.tile([128, 1], F32)
    nc.gpsimd.memset(ones_col[:], 1.0)
    ones_row = const.tile([1, 128], F32)
    nc.gpsimd.memset(ones_row[:], 1.0)
    ones_row16 = const.tile([1, 128], F16)
    nc.gpsimd.memset(ones_row16[:], 1.0)

    # ---------------- P1: x load + per-token absmax -------------------------
    # x DMAs lead the queues; weight-scale shards stream via the DVE queue so
    # the AllGather can fire ~12us in, overlapped with the absmax pass.
    xT = fr.tile([128, NCT, T], F16)
    xTr = io["xT16"].rearrange("(ct p) t -> p ct t", p=128)
    accA = fr.tile([128, T], F16)
    accB = fr.tile([128, T], F16)
    asums = fr.tile([128, 4], F32)
    for w in range(4):
        wsh = fr.tile([128, 1, C], F32, tag="wsh", bufs=4, name=f"wsh_{w}")
        nc.scalar.dma_start(wsh[:], io["Wshards"][:, w:w + 1])
        nc.vector.tensor_reduce(asums[:, w:w + 1], wsh[:, 0], axis=AX.X,
                                op=OP.add, apply_absolute_value=True)
    for ct in range(NCT):
        eng = nc.sync if ct % 2 == 0 else nc.scalar
        eng.dma_start(xT[:, ct], xTr[:, ct])
    part_ps = ppa.tile([1, 4], F32, tag="pp_small")
    nc.tensor.matmul(part_ps[:], ones_col[:], asums[:], start=True, stop=True)
    part_sb = fr.tile([1, 4], F32)
    nc.vector.tensor_copy(part_sb[:], part_ps[:])
    nc.gpsimd.dma_start(ag_in[:], part_sb[:])
    for ct in range(NCT):
        acc, veng = ((accA, nc.vector), (accB, nc.gpsimd))[ct % 2]
        if ct < 2:
            veng.tensor_scalar(acc[:], xT[:, ct], 0.0, None, op0=OP.abs_max)
        else:
            veng.tensor_tensor(acc[:], acc[:], xT[:, ct], op=OP.abs_max)
    acc = accA
    nc.vector.tensor_tensor(acc[:], accA[:], accB[:], op=OP.abs_max)
    # rope tables / J arrive behind x on the Act queue (needed ~20us in)
    t1 = const.tile([128, T], F16)
    t2 = const.tile([128, T], F16)
    nc.scalar.dma_start(t1[:], io["ropeT1"][:])
    nc.scalar.dma_start(t2[:], io["ropeT2"][:])
    nc.scalar.dma_start(jt[:], io["ropeJT"][:])
    # q/k/v weight slices queue behind x on the sync queue
    wsts = {}
    for wn in ("Wq", "Wk", "Wv"):
        wst = wl.tile([128, NCT, HD], F32, tag="wst", bufs=3, name=f"wst_{wn}")
        nc.sync.dma_start(wst[:], io[wn + "Ts"].rearrange("(ct p) o -> p ct o", p=128))
        wsts[wn] = wst

    # per-token channel max, broadcast across partitions in one gpsimd op
    mxb = fr.tile([128, T], F32)
    nc.gpsimd.partition_all_reduce(mxb[:], acc[:], 128, bass_isa.ReduceOp.absmax)
    # AllGather of the four weight |.| partial sums (fires early, overlapped)
    nc.gpsimd.collective_compute(
        "AllGather", OP.bypass, replica_groups=[list(range(NCORES))],
        ins=[ag_in.opt()], outs=[ag_out.opt()])

    # ---------------- P1b: per-token scales (all wide ops) ------------------
    sc_b = mxb   # in-place: becomes the dequant scale, broadcast
    nc.vector.tensor_scalar(sc_b[:], mxb[:], 1e-5, 1.0 / 127.0,
                            op0=OP.max, op1=OP.mult)
    stb = fr.tile([128, T], F32)
    nc.vector.reciprocal(stb[:], sc_b[:])
    scb16 = fr.tile([128, T], F16)
    nc.gpsimd.tensor_copy(scb16[:], sc_b[:])
    t1s = sb.tile([128, T], F16)
    t2s = sb.tile([128, T], F16)
    nc.vector.tensor_tensor(t1s[:], t1[:], scb16[:], op=OP.mult)
    nc.gpsimd.tensor_tensor(t2s[:], t2[:], scb16[:], op=OP.mult)

    # token-major v-copy scale sv[p, tt] = sc(token 128*tt+p) * sw_v, via
    # transposing the (all-rows-identical) broadcast back per token tile

    # quantize: xq8 int8 grid == fp8 a + fp8 b exactly
    a8 = fr.tile([128, NCT, T], F8)
    b8 = fr.tile([128, NCT, T], F8)
    for ct in range(NCT):
        xq8 = fr.tile([128, T], I8, tag="xq8", bufs=2, name=f"xq8_{ct}")
        nc.vector.tensor_tensor(xq8[:], xT[:, ct], stb[:], op=OP.mult)
        nc.scalar.activation(a8[:, ct], xq8[:], ACTF.Copy)
        beng = nc.gpsimd if ct % 2 == 0 else nc.vector
        beng.tensor_tensor(b8[:, ct], xq8[:], a8[:, ct], op=OP.subtract)

    # weight-scale finalization (AllGather landed long ago; emitted here so
    # its waits never park in front of the quant wave)
    gath = fr.tile([8, 4], F32)
    nc.scalar.dma_start(gath[:], ag_out[:])
    swc = {}
    inv_s = {}
    tot_ps = ppa.tile([1, 4], F32, tag="pp_small", name="tot_ps")
    nc.tensor.matmul(tot_ps[:], ones_col[0:8], gath[:], start=True, stop=True)
    sw_row = fr.tile([1, 4], F32)
    nc.vector.tensor_scalar(sw_row[:], tot_ps[:], 1.0 / (C * C), 1e-5,
                            op0=OP.mult, op1=OP.max)
    for wi, wn in enumerate(("Wq", "Wk", "Wv", "Wo")):
        swb_ps = ppa.tile([128, 1], F32, tag="pp_small2", name=f"swb_{wn}")
        nc.tensor.matmul(swb_ps[:], ones_row[:], sw_row[:, wi:wi + 1],
                         start=True, stop=True)
        c_ = sb.tile([128, 1], F32, name=f"swc_{wn}")
        nc.vector.tensor_copy(c_[:], swb_ps[:])
        swc[wn] = c_
        iv = sb.tile([128, 1], F32, name=f"invs_{wn}")
        nc.vector.reciprocal(iv[:], c_[:])
        inv_s[wn] = iv
    svsc = fr.tile([128, NTT, 1], F32)
    for tt in range(NTT):
        trs = ppa.tile([128, 128], F16, tag="trs", bufs=1, name=f"trs_{tt}")
        nc.tensor.transpose(trs[:], scb16[:, 128 * tt:128 * (tt + 1)], id16[:])
        nc.vector.tensor_copy(svsc[:, tt], trs[:, 0:1])
    sv = fr.tile([128, NTT, 1], F32)
    nc.vector.tensor_scalar(sv[:], svsc[:], swc["Wv"][:], None, op0=OP.mult)

    # ---------------- P2: ternarize weight slices to fp8 --------------------
    w8 = {}
    for i, wn in enumerate(("Wq", "Wk", "Wv")):
        wi8 = fr.tile([128, NCT, HD], I8, tag="wi8", bufs=2, name=f"wi8_{wn}")
        e1, e2 = (nc.vector, nc.gpsimd) if i % 2 == 0 else (nc.gpsimd, nc.vector)
        e1.tensor_scalar(wi8[:], wsts[wn][:], inv_s[wn][:], None, op0=OP.mult)
        wf = sb.tile([128, NCT, HD], F8, name=f"w8_{wn}")
        e2.tensor_scalar(wf[:], wi8[:], 1, -1, op0=OP.min, op1=OP.max)
        w8[wn] = wf

    # ---------------- P3: projections --------------------------------------
    psA.close()
    psB = ExitStack()
    ppb = psB.enter_context(tc.tile_pool(name="ppb", bufs=1, space="PSUM"))

    q_sb = sb.tile([128, 2, T], F16)
    k_sb = sb.tile([128, 2, T], F16)
    for name, dst in (("Wq", q_sb), ("Wk", k_sb)):
        for p in range(2):
            for ch in range(NCH):
                tsl = slice(CH * ch, CH * (ch + 1))
                mm = ppb.tile([128, CH], F32, tag="mmq", bufs=2,
                              name=f"mm_{name}{p}{ch}")
                for ctp in range(NCP):
                    for si, src in enumerate((a8, b8)):
                        nc.tensor.matmul(
                            mm[:], w8[name][:, 2 * ctp:2 * ctp + 2, 128 * p:128 * (p + 1)],
                            src[:, 2 * ctp:2 * ctp + 2, tsl],
                            start=(ctp == 0 and si == 0),
                            stop=(ctp == NCP - 1 and si == 1), perf_mode=DR)
                raw = fr.tile([128, CH], F16, tag="raw", bufs=2,
                              name=f"raw_{name}{p}{ch}")
                nc.scalar.activation(raw[:], mm[:], ACTF.Copy)
                jq = ppb.tile([128, CH], F32, tag="jq", bufs=2,
                              name=f"jq_{name}{p}{ch}")
                nc.tensor.matmul(jq[:], jt[:], raw[:], start=True, stop=True)
                p1 = fr.tile([128, CH], F16, tag="p1", bufs=2,
                             name=f"p1_{name}{p}{ch}")
                nc.vector.tensor_tensor(p1[:], mm[:], t1s[:, tsl], op=OP.mult)
                p2t = fr.tile([128, CH], F16, tag="p2", bufs=2,
                              name=f"p2_{name}{p}{ch}")
                nc.gpsimd.tensor_tensor(p2t[:], jq[:], t2s[:, tsl], op=OP.mult)
                nc.vector.tensor_tensor(dst[:, p, tsl], p1[:], p2t[:], op=OP.add)

    v_sb = sb.tile([128, NTT, HPC, 65], F16)
    nc.gpsimd.memset(v_sb[:, :, :, 64:65], 1.0)
    for tt in range(NTT):
        mmv = ppb.tile([128, HD], F32, tag="mmv", bufs=3, name=f"mmv_{tt}")
        for ctp in range(NCP):
            for si, src in enumerate((a8, b8)):
                nc.tensor.matmul(
                    mmv[:], src[:, 2 * ctp:2 * ctp + 2, 128 * tt:128 * (tt + 1)],
                    w8["Wv"][:, 2 * ctp:2 * ctp + 2, :],
                    start=(ctp == 0 and si == 0),
                    stop=(ctp == NCP - 1 and si == 1), perf_mode=DR)
        nc.scalar.activation(v_sb[:, tt, :, 0:64],
                             mmv[:].rearrange("p (h d) -> p h d", h=HPC),
                             ACTF.Copy, scale=sv[:, tt])

    # ---------------- P4: attention + per-pair resharding -------------------
    expsc = sb.tile([128, 1], F32)
    nc.vector.tensor_tensor(expsc[:], swc["Wq"][:], swc["Wk"][:], op=OP.mult)
    nc.vector.tensor_scalar(expsc[:], expsc[:], 1.0 / np.sqrt(np.float64(D)), None,
                            op0=OP.mult)
    swco = sb.tile([128, 1], F32)
    nc.vector.tensor_copy(swco[:], swc["Wo"][:])
    osiv = sb.tile([128, 1], F32)
    nc.vector.tensor_copy(osiv[:], inv_s["Wo"][:])

    front.close()
    psB.close()
    expp = es.enter_context(tc.tile_pool(name="expp", bufs=1))
    tail = es.enter_context(tc.tile_pool(name="tail", bufs=1))
    psC = ExitStack()
    ppc = psC.enter_context(tc.tile_pool(name="ppc", bufs=1, space="PSUM"))

    y_sb = sb.tile([128, NTT, HPC, D], F16)
    # channel-major resharded y (built during attention from half arrivals)
    ycT = tail.tile([128, NCT, OT], F16)
    accY = tail.tile([128, OT], F16)
    nc.gpsimd.memset(accY[:], 0.0)

    # Wo loads + ternarize overlap attention (DMA and DVE are mostly idle)
    wo8 = tail.tile([128, NCT, C], F8)
    woR = io["WoT"].rearrange("(ct p) o -> p ct o", p=128)
    for chunk in range(4):
        wst = tail.tile([128, 2, C], F32, tag="woc", bufs=2, name=f"woc_{chunk}")
        nc.sync.dma_start(wst[:], woR[:, 2 * chunk:2 * chunk + 2])
        wi8o = tail.tile([128, 2, C], I8, tag="woi", bufs=2, name=f"woi_{chunk}")
        nc.vector.tensor_scalar(wi8o[:], wst[:], osiv[:], None, op0=OP.mult)
        nc.vector.tensor_scalar(wo8[:, 2 * chunk:2 * chunk + 2], wi8o[:], 1, -1,
                                op0=OP.min, op1=OP.max)

    def head_thunks(p, e):
        """List of emission thunks for head (p,e): one per score group plus
        one per epilogue block; interleaving two heads' thunks keeps the
        exp pipe and PE continuously fed."""
        h = 2 * p + e
        thunks = []
        state = {"first": True, "kt": 0, "yaug": None}

        def mk_yaug(jb):
            def f():
                state["yaug"] = ppc.tile([65, QB], F32, tag=f"yaug{e}",
                                         name=f"yaug{h}{jb}")
                state["first"] = True
            return f

        def av(egrp, esl, csl, stop):
            nc.tensor.matmul(state["yaug"][:, csl], v_sb[:, state["kt"], h, :],
                             egrp[:, esl], start=state["first"], stop=stop,
                             skip_group_check=True)
            state["first"] = False

        def mk_full(jb, fg):
            def f():
                sgrp = ppc.tile([128, 2 * QB], F32, tag="sgrp", bufs=2,
                                name=f"sgrp{h}{jb}{fg}")
                for m in range(2):
                    kt = fg * 2 + m
                    nc.tensor.matmul(
                        sgrp[:, QB * m:QB * (m + 1)],
                        k_sb[64 * e:64 * (e + 1), p, 128 * kt:128 * (kt + 1)],
                        q_sb[64 * e:64 * (e + 1), p, QB * jb:QB * (jb + 1)],
                        start=True, stop=True, tile_position=(64 * e, 0))
                egrp = expp.tile([128, 2 * QB], F16, tag=f"egrp{e}", bufs=3,
                                 name=f"egrp{h}{jb}{fg}")
                nc.scalar.activation(egrp[:], sgrp[:], ACTF.Exp, scale=expsc[:])
                for m in range(2):
                    state["kt"] = fg * 2 + m
                    av(egrp, slice(QB * m, QB * (m + 1)), slice(0, QB),
                       stop=False)
            return f

        def mk_diag(jb, dpair):
            def f():
                widths = [QB - KT * (2 * dpair), QB - KT * (2 * dpair + 1)]
                wtot = sum(widths)
                offs = [0, widths[0]]
                sgrp = ppc.tile([128, wtot], F32, tag="sgrp", bufs=2,
                                name=f"sgrpd{h}{jb}{dpair}")
                for ii in range(2):
                    i = 2 * dpair + ii
                    kt = 4 * jb + i
                    nc.tensor.matmul(
                        sgrp[:, offs[ii]:offs[ii] + widths[ii]],
                        k_sb[64 * e:64 * (e + 1), p, 128 * kt:128 * (kt + 1)],
                        q_sb[64 * e:64 * (e + 1), p,
                             QB * jb + KT * i:QB * (jb + 1)],
                        start=True, stop=True, tile_position=(64 * e, 0))
                egrp = expp.tile([128, wtot], F16, tag=f"egrp{e}", bufs=3,
                                 name=f"egrpd{h}{jb}{dpair}")
                nc.scalar.activation(egrp[:], sgrp[:], ACTF.Exp, scale=expsc[:])
                for ii in range(2):
                    # staircase mask on the first 128 columns of each slice
                    nc.gpsimd.affine_select(
                        out=egrp[:, offs[ii]:offs[ii] + KT],
                        in_=egrp[:, offs[ii]:offs[ii] + KT],
                        compare_op=OP.is_ge, fill=0.0,
                        base=0, pattern=[[1, KT]], channel_multiplier=-1)
                for ii in range(2):
                    i = 2 * dpair + ii
                    state["kt"] = 4 * jb + i
                    av(egrp, slice(offs[ii], offs[ii] + widths[ii]),
                       slice(KT * i, QB),
                       stop=(dpair == 1 and ii == 1))
            return f

        def mk_epi(jb):
            def f():
                yaug16 = expp.tile([65, QB], F16, tag=f"yaug16_{e}", bufs=2,
                                   name=f"yaug16{h}{jb}")
                nc.vector.tensor_copy(yaug16[:], state["yaug"][:])
                for chn in range(QB // 128):
                    trp = ppc.tile([128, 512], F16, tag="trx", bufs=2,
                                   name=f"trp{h}{jb}{chn}")
                    nc.tensor.transpose(trp[:, 0:65],
                                        yaug16[:, 128 * chn:128 * (chn + 1)],
                                        id16[0:65, 0:65])
                    rec = expp.tile([128, 1], F32, tag=f"rec{e}", bufs=2,
                                    name=f"rec{h}{jb}{chn}")
                    nc.vector.reciprocal(rec[:], trp[:, 64:65])
                    nc.vector.tensor_scalar(
                        y_sb[:, 4 * jb + chn, h, :], trp[:, 0:64],
                        rec[:], None, op0=OP.mult)
            return f

        for jb in range(NQB):
            thunks.append(mk_yaug(jb))
            for fg in range(2 * jb):
                thunks.append(mk_full(jb, fg))
            for dpair in range(2):
                thunks.append(mk_diag(jb, dpair))
            thunks.append(mk_epi(jb))
        return thunks

    def attention_pair(p):
        for a, b in zip_longest(head_thunks(p, 0), head_thunks(p, 1)):
            if a is not None:
                a()
            if b is not None:
                b()

    def send_half(ph):
        a2a_in, a2a_out = a2a[ph]
        for d in range(NCORES):
            nc.sync.dma_start(
                a2a_in[d].rearrange("(p t h dd) -> p t h dd", p=128, t=2, h=2),
                y_sb[:, 2 * d:2 * d + 2, 2 * ph:2 * ph + 2, :])
        nc.gpsimd.collective_compute(
            "AllToAll", OP.bypass, replica_groups=[list(range(NCORES))],
            ins=[a2a_in.opt()], outs=[a2a_out.opt()])

    def recv_half(ph):
        """Unpack half ph into channel-major ycT and fold into accY.

        Source s holds global heads {4*(s%4)+2*ph, +1}, i.e. channels
        [256*(s%4)+128*ph, +128) -> ct = 2*(s%4) + ph, lanes 0..127.
        """
        _, a2a_out = a2a[ph]
        for s in range(NCORES):
            ct = 2 * (s % 4) + ph
            yarr = tail.tile([128, 2, 2, D], F16, tag="yarr", bufs=4,
                             name=f"yarr{ph}{s}")
            nc.sync.dma_start(
                yarr[:],
                a2a_out[s].rearrange("(p t h dd) -> p t h dd", p=128, t=2, h=2))
            try8 = ppc.tile([128, 512], F16, tag="trx", bufs=2,
                            name=f"try{ph}{s}")
            for tl in range(2):
                for hh in range(2):
                    nc.tensor.transpose(
                        try8[64 * hh:64 * (hh + 1), 128 * tl:128 * (tl + 1)],
                        yarr[:, tl, hh], id16[:])
            tt_loc = 2 * (s // 4)
            csl = slice(128 * tt_loc, 128 * (tt_loc + 2))
            eng = (nc.vector, nc.gpsimd)[s % 2]
            eng.tensor_copy(ycT[:, ct, csl], try8[:, 0:256])
            eng.tensor_tensor(accY[:, csl], accY[:, csl], try8[:, 0:256],
                              op=OP.abs_max)

    attention_pair(0)
    send_half(0)
    attention_pair(1)
    recv_half(0)
    send_half(1)
    recv_half(1)

    # ---------------- P5: output quant + projection -------------------------
    psC.close()
    psD = ExitStack()
    ppd = psD.enter_context(tc.tile_pool(name="ppd", bufs=1, space="PSUM"))

    mxyb = tail.tile([128, OT], F32)
    nc.gpsimd.partition_all_reduce(mxyb[:], accY[:], 128, bass_isa.ReduceOp.absmax)
    scyb = mxyb   # in-place
    nc.vector.tensor_scalar(scyb[:], mxyb[:], 1e-5, 1.0 / 127.0,
                            op0=OP.max, op1=OP.mult)
    styb = tail.tile([128, OT], F32)
    nc.vector.reciprocal(styb[:], scyb[:])
    scyb16 = tail.tile([128, OT], F16)
    nc.gpsimd.tensor_copy(scyb16[:], scyb[:])
    osc = tail.tile([128, OTT, 1], F32)
    for tt in range(OTT):
        trso = ppd.tile([128, 128], F16, tag="trso", bufs=1, name=f"trso_{tt}")
        nc.tensor.transpose(trso[:], scyb16[:, 128 * tt:128 * (tt + 1)], id16[:])
        nc.vector.tensor_scalar(osc[:, tt], trso[:, 0:1], swco[:], None,
                                op0=OP.mult)

    ya8 = tail.tile([128, NCT, OT], F8)
    yb8 = tail.tile([128, NCT, OT], F8)
    for ct in range(NCT):
        yq8 = tail.tile([128, OT], I8, tag="yq8", bufs=2, name=f"yq8_{ct}")
        nc.vector.tensor_tensor(yq8[:], ycT[:, ct], styb[:], op=OP.mult)
        nc.scalar.activation(ya8[:, ct], yq8[:], ACTF.Copy)
        nc.gpsimd.tensor_tensor(yb8[:, ct], yq8[:], ya8[:, ct], op=OP.subtract)

    outR = io["out_slice"].rearrange("(tt p) c -> p tt c", p=128)
    for tt in range(OTT):
        pso = ppd.tile([128, C], F32, tag="pso", bufs=2, name=f"pso_{tt}")
        for ctp in range(NCP):
            for si, srcT in enumerate((ya8, yb8)):
                nc.tensor.matmul(
                    pso[:], srcT[:, 2 * ctp:2 * ctp + 2, 128 * tt:128 * (tt + 1)],
                    wo8[:, 2 * ctp:2 * ctp + 2, :],
                    start=(ctp == 0 and si == 0),
                    stop=(ctp == NCP - 1 and si == 1), perf_mode=DR)
        outt = tail.tile([128, C], F32, tag="outt", bufs=2, name=f"outt_{tt}")
        nc.scalar.activation(outt[:], pso[:], ACTF.Copy, scale=osc[:, tt])
        nc.sync.dma_start(outR[:, tt], outt[:])
    psD.close()
    es.close()


def kernel(x, Wq, Wk, Wv, Wo, _trace=False):
    x = np.ascontiguousarray(np.asarray(x, np.float32))
    if "nc" not in _CACHE:
        _CACHE["nc"] = build_program()
    nc = _CACHE["nc"]
    t1, t2 = _host_tables()
    jth = _host_jt()
    ws = {"Wq": np.asarray(Wq, np.float32), "Wk": np.asarray(Wk, np.float32),
          "Wv": np.asarray(Wv, np.float32), "Wo": np.asarray(Wo, np.float32)}
    wstack = np.stack([ws["Wq"], ws["Wk"], ws["Wv"], ws["Wo"]], axis=0)
    woT = np.ascontiguousarray(ws["Wo"].T)
    in_maps = []
    for c in range(NCORES):
        b, j = c // 4, c % 4
        in_maps.append({
            "xT16": np.ascontiguousarray(x[b].T.astype(np.float16)),
            "Wshards": np.ascontiguousarray(
                wstack[:, 128 * c:128 * (c + 1), :].transpose(1, 0, 2)),
            "WqTs": np.ascontiguousarray(ws["Wq"][HD * j:HD * (j + 1), :].T),
            "WkTs": np.ascontiguousarray(ws["Wk"][HD * j:HD * (j + 1), :].T),
            "WvTs": np.ascontiguousarray(ws["Wv"][HD * j:HD * (j + 1), :].T),
            "WoT": woT,
            "ropeT1": t1, "ropeT2": t2, "ropeJT": jth,
        })
    res = run_bass_kernel_spmd(nc, in_maps, list(range(NCORES)), trace=_trace)
    out = np.zeros((B, T, C), np.float32)
    for c in range(NCORES):
        o = np.asarray(res.results[c]["out_slice"])
        out[0, 256 * c:256 * (c + 1)] = o[0:256]
        out[1, 256 * c:256 * (c + 1)] = o[256:512]
    if _trace:
        return out, res
    return out


# revision 21
# speedup vs baseline: 1.1371x; 1.0944x over previous
"""Trainium2 Bass kernel for nn_CausalSelfAttention_52905407152466.

BitNet-style causal self-attention, distributed over 8 NeuronCores with an
instance-parallel (batch x head-group) sharding that needs NO collective
before attention:

  - core c owns batch b=c//4 and heads {4j..4j+3} with j=c%4.  It receives
    its batch's x (transposed, fp16) and the column slices of Wq/Wk/Wv for
    its heads, computes q,k,v for all 2048 tokens of its batch locally, and
    runs causal attention for its 4 heads.
  - the ternary weight scales (mean|W|) need the full matrices; each core
    reduces a 1/8 row shard of each W and a 128-byte AllGather combines the
    partial sums (fully overlapped with the x pipeline).
  - y reshards to token-sharded via FOUR quarter AllToAlls, one per head,
    each issued as soon as that head's attention finishes so only the last
    quarter's transfer is exposed.  Arriving quarters are transposed to
    channel-major during attention, so the tail is just quant + Wo matmuls.

Numerics: act_quant int8 values split EXACTLY into two fp8e4m3 operands
(a=fp8(v), b=v-a with |b|<=4; both exact), and ternary weights are exact in
fp8, so every projection runs as DoubleRow fp8 matmuls (2x fp16 throughput)
while reproducing the reference int8xternary products exactly (fp32 psum).
Per-token activation-quant scales are folded into the rope tables (q,k),
the exp scale (sw_q*sw_k/sqrt(D)), the v psum copy (sc*sw_v) and the output
copy (scy*sw_o).  Attention runs in fp16 with the ones-column-in-V
normalizer; diagonal score tiles are processed ragged (only the visible
columns are computed/exponentiated) with a single 128-wide affine_select
boundary mask per tile.
"""

import numpy as np

import concourse.bacc as bacc
import concourse.mybir as mybir
import concourse.tile as tile
from concourse import bass_isa
from concourse.bass_utils import run_bass_kernel_spmd
from concourse.masks import make_identity

F32 = mybir.dt.float32
F16 = mybir.dt.float16
F8 = mybir.dt.float8e4
I8 = mybir.dt.int8
AX = mybir.AxisListType
OP = mybir.AluOpType
ACTF = mybir.ActivationFunctionType
DR = mybir.MatmulPerfMode.DoubleRow

NCORES = 8
B, T, C = 2, 2048, 1024
H, D = 16, 64
HPC = 4                     # heads per core
HD = HPC * D                # 256 projection channels per core
NCT = C // 128              # 8 channel tiles
NCP = NCT // 2              # 4 channel-tile pairs (DoubleRow)
NTT = T // 128              # 16 token tiles per batch
QB = 512                    # query block
NQB = T // QB               # 4
KT = 128                    # key tile
OT = 512                    # owned output tokens per core (256 per batch)
OTT = OT // 128             # 4
CH = 512                    # q/k projection token chunk
NCH = T // CH               # 4
QSZ = 128 * 2 * D           # a2a quarter slot elems: 128p x 2 tiles x 64
ROPE_BASE = 10000.0

_CACHE = {}


def _host_tables():
    pos = np.arange(T, dtype=np.float64)
    inv = 1.0 / (ROPE_BASE ** (np.arange(0, D, 2, dtype=np.float64) / D))
    ang = pos[None, :] * inv[:, None]              # [32, T]
    cos = np.cos(ang).astype(np.float32).astype(np.float16)
    sin = np.sin(ang).astype(np.float32).astype(np.float16)
    t1 = np.concatenate([cos, cos, cos, cos], axis=0)
    t2 = np.concatenate([sin, sin, sin, sin], axis=0)
    return t1.astype(np.float16), t2.astype(np.float16)


def _host_jt():
    i32 = np.eye(32, dtype=np.float16)
    z = np.zeros((32, 32), np.float16)
    j64 = np.block([[z, -i32], [i32, z]])     # J: Jq[0:32] = -q[32:64]; Jq[32:64] = q[0:32]
    jt = np.block([[j64.T, np.zeros((64, 64), np.float16)],
                   [np.zeros((64, 64), np.float16), j64.T]])
    return jt.astype(np.float16)


def build_program():
    nc = bacc.Bacc("TRN2", target_bir_lowering=False, debug=False,
                   num_devices=NCORES)
    io = {}

    def inp(name, shape, dtype=F32):
        io[name] = nc.declare_dram_parameter(name, list(shape), dtype, isOutput=False)
        return io[name]

    def outp(name, shape, dtype=F32):
        io[name] = nc.declare_dram_parameter(name, list(shape), dtype, isOutput=True)
        return io[name]

    inp("xT16", (C, T), F16)
    inp("Wshards", (128, 4, C))
    for n in ("Wq", "Wk", "Wv"):
        inp(n + "Ts", (C, HD))
    inp("WoT", (C, C))
    inp("ropeT1", (128, T), F16)
    inp("ropeT2", (128, T), F16)
    inp("ropeJT", (128, 128), F16)
    outp("out_slice", (OT, C))

    with tile.TileContext(nc) as tc:
        with tc.tile_pool(name="dram", bufs=1, space="DRAM") as dram:
            ag_in = dram.tile([1, 4], F32)
            ag_out = dram.tile([8, 4], F32)
            a2a = [(dram.tile([NCORES, 2 * QSZ], F16, name=f"a2a_in{q}"),
                    dram.tile([NCORES, 2 * QSZ], F16, name=f"a2a_out{q}"))
                   for q in range(2)]
            _build_body(nc, tc, io, ag_in, ag_out, a2a)
    nc.compile()
    return nc


def _build_body(nc, tc, io, ag_in, ag_out, a2a):
    from contextlib import ExitStack
    from itertools import zip_longest
    es = ExitStack()
    const = es.enter_context(tc.tile_pool(name="const", bufs=1))
    sb = es.enter_context(tc.tile_pool(name="sb", bufs=1))
    front = ExitStack()
    fr = front.enter_context(tc.tile_pool(name="fr", bufs=1))
    wl = front.enter_context(tc.tile_pool(name="wl", bufs=1))
    psA = ExitStack()
    ppa = psA.enter_context(tc.tile_pool(name="ppa", bufs=1, space="PSUM"))

    # ---------------- constants -------------------------------------------
    id16 = const.tile([128, 128], F16)
    make_identity(nc, id16[:])
    id8 = const.tile([128, 128], F8)
    make_identity(nc, id8[:])
    jt = const.tile([128, 128], F16)
    ones_col = const.tile([128, 1], F32)
    nc.gpsimd.memset(ones_col[:], 1.0)
    ones_row = const.tile([1, 128], F32)
    nc.gpsimd.memset(ones_row[:], 1.0)
    ones_row16 = const.tile([1, 128], F16)
    nc.gpsimd.memset(ones_row16[:], 1.0)

    # ---------------- P1: x load + per-token absmax -------------------------
    # x DMAs lead the queues; weight-scale shards stream via the DVE queue so
    # the AllGather can fire ~12us in, overlapped with the absmax pass.
    xT = fr.tile([128, NCT, T], F16)
    xTr = io["xT16"].rearrange("(ct p) t -> p ct t", p=128)
    accA = fr.tile([128, T], F16)
    accB = fr.tile([128, T], F16)
    asums = fr.tile([128, 4], F32)
    for ct in range(NCT):
        eng = nc.sync if ct % 2 == 0 else nc.scalar
        eng.dma_start(xT[:, ct], xTr[:, ct])
        acc, veng = ((accA, nc.vector), (accB, nc.gpsimd))[ct % 2]
        if ct < 2:
            veng.tensor_scalar(acc[:], xT[:, ct], 0.0, None, op0=OP.abs_max)
        else:
            veng.tensor_tensor(acc[:], acc[:], xT[:, ct], op=OP.abs_max)
    acc = accA
    nc.vector.tensor_tensor(acc[:], accA[:], accB[:], op=OP.abs_max)
    # weight-scale shards: |.| row sums via Abs activation with accumulate,
    # keeping the DVE free for the scale chain / quant wave
    for w in range(4):
        wsh = fr.tile([128, 1, C], F32, tag="wsh", bufs=4, name=f"wsh_{w}")
        nc.scalar.dma_start(wsh[:], io["Wshards"][:, w:w + 1])
        junk = fr.tile([128, C], F16, tag="junk", bufs=1, name=f"junk_{w}")
        nc.scalar.activation(junk[:], wsh[:, 0], ACTF.Abs,
                             accum_out=asums[:, w:w + 1])
    part_ps = ppa.tile([1, 4], F32, tag="pp_small")
    nc.tensor.matmul(part_ps[:], ones_col[:], asums[:], start=True, stop=True)
    part_sb = fr.tile([1, 4], F32)
    nc.vector.tensor_copy(part_sb[:], part_ps[:])
    nc.gpsimd.dma_start(ag_in[:], part_sb[:])
    # rope tables / J behind the shards on the Act queue
    t1 = const.tile([128, T], F16)
    t2 = const.tile([128, T], F16)
    nc.scalar.dma_start(t1[:], io["ropeT1"][:])
    nc.scalar.dma_start(t2[:], io["ropeT2"][:])
    nc.scalar.dma_start(jt[:], io["ropeJT"][:])
    # q/k/v weight slices queue behind x on the sync queue
    wsts = {}
    for wn in ("Wq", "Wk", "Wv"):
        wst = wl.tile([128, NCT, HD], F32, tag="wst", bufs=2, name=f"wst_{wn}")
        nc.sync.dma_start(wst[:], io[wn + "Ts"].rearrange("(ct p) o -> p ct o", p=128))
        wsts[wn] = wst

    # per-token channel max, broadcast across partitions in one gpsimd op
    mxb = fr.tile([128, T], F32)
    nc.gpsimd.partition_all_reduce(mxb[:], acc[:], 128, bass_isa.ReduceOp.absmax)
    # AllGather of the four weight |.| partial sums (fires early, overlapped)
    nc.gpsimd.collective_compute(
        "AllGather", OP.bypass, replica_groups=[list(range(NCORES))],
        ins=[ag_in.opt()], outs=[ag_out.opt()])

    # ---------------- P1b: per-token scales (all wide ops) ------------------
    sc_b = mxb   # in-place: becomes the dequant scale, broadcast
    nc.vector.tensor_scalar(sc_b[:], mxb[:], 1e-5, 1.0 / 127.0,
                            op0=OP.max, op1=OP.mult)
    stb = fr.tile([128, T], F32)
    nc.vector.reciprocal(stb[:], sc_b[:])
    scb16 = fr.tile([128, T], F16)
    nc.gpsimd.tensor_copy(scb16[:], sc_b[:])
    t1s = sb.tile([128, T], F16)
    t2s = sb.tile([128, T], F16)
    nc.vector.tensor_tensor(t1s[:], t1[:], scb16[:], op=OP.mult)
    nc.gpsimd.tensor_tensor(t2s[:], t2[:], scb16[:], op=OP.mult)

    # token-major v-copy scale sv[p, tt] = sc(token 128*tt+p) * sw_v, via
    # transposing the (all-rows-identical) broadcast back per token tile

    # quantize: xq8 int8 grid == fp8 a + fp8 b exactly
    a8 = fr.tile([128, NCT, T], F8)
    b8 = fr.tile([128, NCT, T], F8)
    for ct in range(NCT):
        xq8 = fr.tile([128, T], I8, tag="xq8", bufs=3, name=f"xq8_{ct}")
        qeng = nc.gpsimd if ct in (2, 5) else nc.vector
        qeng.tensor_tensor(xq8[:], xT[:, ct], stb[:], op=OP.mult)
        nc.scalar.activation(a8[:, ct], xq8[:], ACTF.Copy)
        beng = nc.gpsimd if ct % 2 == 0 else nc.vector
        beng.tensor_tensor(b8[:, ct], xq8[:], a8[:, ct], op=OP.subtract)

    # weight-scale finalization (AllGather landed long ago; emitted here so
    # its waits never park in front of the quant wave)
    gath = fr.tile([8, 4], F32)
    nc.sync.dma_start(gath[:], ag_out[:])
    swc = {}
    inv_s = {}
    tot_ps = ppa.tile([1, 4], F32, tag="pp_small", name="tot_ps")
    nc.tensor.matmul(tot_ps[:], ones_col[0:8], gath[:], start=True, stop=True)
    sw_row = fr.tile([1, 4], F32)
    nc.vector.tensor_scalar(sw_row[:], tot_ps[:], 1.0 / (C * C), 1e-5,
                            op0=OP.mult, op1=OP.max)
    for wi, wn in enumerate(("Wq", "Wk", "Wv", "Wo")):
        swb_ps = ppa.tile([128, 1], F32, tag="pp_small2", name=f"swb_{wn}")
        nc.tensor.matmul(swb_ps[:], ones_row[:], sw_row[:, wi:wi + 1],
                         start=True, stop=True)
        c_ = sb.tile([128, 1], F32, name=f"swc_{wn}")
        nc.vector.tensor_copy(c_[:], swb_ps[:])
        swc[wn] = c_
        iv = sb.tile([128, 1], F32, name=f"invs_{wn}")
        nc.vector.reciprocal(iv[:], c_[:])
        inv_s[wn] = iv
    svsc = fr.tile([128, NTT, 1], F32)
    for tt in range(NTT):
        trs = ppa.tile([128, 128], F16, tag="trs", bufs=1, name=f"trs_{tt}")
        nc.tensor.transpose(trs[:], scb16[:, 128 * tt:128 * (tt + 1)], id16[:])
        nc.vector.tensor_copy(svsc[:, tt], trs[:, 0:1])
    sv = fr.tile([128, NTT, 1], F32)
    nc.vector.tensor_scalar(sv[:], svsc[:], swc["Wv"][:], None, op0=OP.mult)

    # ---------------- P2: ternarize weight slices to fp8 --------------------
    w8 = {}
    for i, wn in enumerate(("Wq", "Wk", "Wv")):
        wi8 = fr.tile([128, NCT, HD], I8, tag="wi8", bufs=2, name=f"wi8_{wn}")
        nc.scalar.activation(wi8[:], wsts[wn][:], ACTF.Copy, scale=inv_s[wn][:])
        wf = sb.tile([128, NCT, HD], F8, name=f"w8_{wn}")
        e2 = (nc.vector, nc.gpsimd, nc.vector)[i]
        e2.tensor_scalar(wf[:], wi8[:], 1, -1, op0=OP.min, op1=OP.max)
        w8[wn] = wf

    # ---------------- P3: projections --------------------------------------
    psA.close()
    psB = ExitStack()
    ppb = psB.enter_context(tc.tile_pool(name="ppb", bufs=1, space="PSUM"))

    q_sb = sb.tile([128, 2, T], F16)
    k_sb = sb.tile([128, 2, T], F16)
    for name, dst in (("Wq", q_sb), ("Wk", k_sb)):
        for p in range(2):
            for ch in range(NCH):
                tsl = slice(CH * ch, CH * (ch + 1))
                mm = ppb.tile([128, CH], F32, tag="mmq", bufs=2,
                              name=f"mm_{name}{p}{ch}")
                for ctp in range(NCP):
                    for si, src in enumerate((a8, b8)):
                        nc.tensor.matmul(
                            mm[:], w8[name][:, 2 * ctp:2 * ctp + 2, 128 * p:128 * (p + 1)],
                            src[:, 2 * ctp:2 * ctp + 2, tsl],
                            start=(ctp == 0 and si == 0),
                            stop=(ctp == NCP - 1 and si == 1), perf_mode=DR)
                raw = fr.tile([128, CH], F16, tag="raw", bufs=2,
                              name=f"raw_{name}{p}{ch}")
                nc.scalar.activation(raw[:], mm[:], ACTF.Copy)
                jq = ppb.tile([128, CH], F32, tag="jq", bufs=2,
                              name=f"jq_{name}{p}{ch}")
                nc.tensor.matmul(jq[:], jt[:], raw[:], start=True, stop=True)
                p1 = fr.tile([128, CH], F16, tag="p1", bufs=2,
                             name=f"p1_{name}{p}{ch}")
                nc.vector.tensor_tensor(p1[:], mm[:], t1s[:, tsl], op=OP.mult)
                p2t = fr.tile([128, CH], F16, tag="p2", bufs=2,
                              name=f"p2_{name}{p}{ch}")
                nc.gpsimd.tensor_tensor(p2t[:], jq[:], t2s[:, tsl], op=OP.mult)
                nc.vector.tensor_tensor(dst[:, p, tsl], p1[:], p2t[:], op=OP.add)

    v_sb = sb.tile([128, NTT, HPC, 65], F16)
    nc.gpsimd.memset(v_sb[:, :, :, 64:65], 1.0)
    for tt in range(NTT):
        mmv = ppb.tile([128, HD], F32, tag="mmv", bufs=3, name=f"mmv_{tt}")
        for ctp in range(NCP):
            for si, src in enumerate((a8, b8)):
                nc.tensor.matmul(
                    mmv[:], src[:, 2 * ctp:2 * ctp + 2, 128 * tt:128 * (tt + 1)],
                    w8["Wv"][:, 2 * ctp:2 * ctp + 2, :],
                    start=(ctp == 0 and si == 0),
                    stop=(ctp == NCP - 1 and si == 1), perf_mode=DR)
        nc.scalar.activation(v_sb[:, tt, :, 0:64],
                             mmv[:].rearrange("p (h d) -> p h d", h=HPC),
                             ACTF.Copy, scale=sv[:, tt])

    # ---------------- P4: attention + per-pair resharding -------------------
    expsc = sb.tile([128, 1], F32)
    nc.vector.tensor_tensor(expsc[:], swc["Wq"][:], swc["Wk"][:], op=OP.mult)
    nc.vector.tensor_scalar(expsc[:], expsc[:], 1.0 / np.sqrt(np.float64(D)), None,
                            op0=OP.mult)
    swco = sb.tile([128, 1], F32)
    nc.vector.tensor_copy(swco[:], swc["Wo"][:])
    osiv = sb.tile([128, 1], F32)
    nc.vector.tensor_copy(osiv[:], inv_s["Wo"][:])

    front.close()
    psB.close()
    expp = es.enter_context(tc.tile_pool(name="expp", bufs=1))
    tail = es.enter_context(tc.tile_pool(name="tail", bufs=1))
    psC = ExitStack()
    ppc = psC.enter_context(tc.tile_pool(name="ppc", bufs=1, space="PSUM"))

    y_sb = sb.tile([128, NTT, HPC, D], F16)
    # channel-major resharded y (built during attention from half arrivals)
    ycT = tail.tile([128, NCT, OT], F16)
    accY = tail.tile([128, OT], F16)
    nc.gpsimd.memset(accY[:], 0.0)

    # Wo loads + ternarize overlap attention (DMA and DVE are mostly idle)
    wo8 = tail.tile([128, NCT, C], F8)
    woR = io["WoT"].rearrange("(ct p) o -> p ct o", p=128)
    for chunk in range(4):
        wst = tail.tile([128, 2, C], F32, tag="woc", bufs=2, name=f"woc_{chunk}")
        nc.sync.dma_start(wst[:], woR[:, 2 * chunk:2 * chunk + 2])
        wi8o = tail.tile([128, 2, C], I8, tag="woi", bufs=2, name=f"woi_{chunk}")
        nc.vector.tensor_scalar(wi8o[:], wst[:], osiv[:], None, op0=OP.mult)
        nc.gpsimd.tensor_scalar(wo8[:, 2 * chunk:2 * chunk + 2], wi8o[:], 1, -1,
                                op0=OP.min, op1=OP.max)

    def head_thunks(p, e):
        """List of emission thunks for head (p,e): one per score group plus
        one per epilogue block; interleaving two heads' thunks keeps the
        exp pipe and PE continuously fed."""
        h = 2 * p + e
        thunks = []
        state = {"first": True, "kt": 0, "yaug": None}

        def mk_yaug(jb):
            def f():
                state["yaug"] = ppc.tile([65, QB], F32, tag=f"yaug{e}",
                                         name=f"yaug{h}{jb}")
                state["first"] = True
            return f

        def av(egrp, esl, csl, stop):
            nc.tensor.matmul(state["yaug"][:, csl], v_sb[:, state["kt"], h, :],
                             egrp[:, esl], start=state["first"], stop=stop,
                             skip_group_check=True)
            state["first"] = False

        def mk_full(jb, fg):
            def f():
                sgrp = ppc.tile([128, 2 * QB], F32, tag="sgrp", bufs=2,
                                name=f"sgrp{h}{jb}{fg}")
                for m in range(2):
                    kt = fg * 2 + m
                    nc.tensor.matmul(
                        sgrp[:, QB * m:QB * (m + 1)],
                        k_sb[64 * e:64 * (e + 1), p, 128 * kt:128 * (kt + 1)],
                        q_sb[64 * e:64 * (e + 1), p, QB * jb:QB * (jb + 1)],
                        start=True, stop=True, tile_position=(64 * e, 0))
                egrp = expp.tile([128, 2 * QB], F16, tag=f"egrp{e}", bufs=3,
                                 name=f"egrp{h}{jb}{fg}")
                nc.scalar.activation(egrp[:], sgrp[:], ACTF.Exp, scale=expsc[:])
                for m in range(2):
                    state["kt"] = fg * 2 + m
                    av(egrp, slice(QB * m, QB * (m + 1)), slice(0, QB),
                       stop=False)
            return f

        def mk_diag(jb, dpair):
            def f():
                widths = [QB - KT * (2 * dpair), QB - KT * (2 * dpair + 1)]
                wtot = sum(widths)
                offs = [0, widths[0]]
                sgrp = ppc.tile([128, wtot], F32, tag="sgrp", bufs=2,
                                name=f"sgrpd{h}{jb}{dpair}")
                for ii in range(2):
                    i = 2 * dpair + ii
                    kt = 4 * jb + i
                    nc.tensor.matmul(
                        sgrp[:, offs[ii]:offs[ii] + widths[ii]],
                        k_sb[64 * e:64 * (e + 1), p, 128 * kt:128 * (kt + 1)],
                        q_sb[64 * e:64 * (e + 1), p,
                             QB * jb + KT * i:QB * (jb + 1)],
                        start=True, stop=True, tile_position=(64 * e, 0))
                egrp = expp.tile([128, wtot], F16, tag=f"egrp{e}", bufs=3,
                                 name=f"egrpd{h}{jb}{dpair}")
                nc.scalar.activation(egrp[:], sgrp[:], ACTF.Exp, scale=expsc[:])
                for ii in range(2):
                    # staircase mask on the first 128 columns of each slice
                    nc.gpsimd.affine_select(
                        out=egrp[:, offs[ii]:offs[ii] + KT],
                        in_=egrp[:, offs[ii]:offs[ii] + KT],
                        compare_op=OP.is_ge, fill=0.0,
                        base=0, pattern=[[1, KT]], channel_multiplier=-1)
                for ii in range(2):
                    i = 2 * dpair + ii
                    state["kt"] = 4 * jb + i
                    av(egrp, slice(offs[ii], offs[ii] + widths[ii]),
                       slice(KT * i, QB),
                       stop=(dpair == 1 and ii == 1))
            return f

        def mk_epi(jb):
            def f():
                yaug16 = expp.tile([65, QB], F16, tag=f"yaug16_{e}", bufs=2,
                                   name=f"yaug16{h}{jb}")
                nc.vector.tensor_copy(yaug16[:], state["yaug"][:])
                for chn in range(QB // 128):
                    trp = ppc.tile([128, 512], F16, tag="trx", bufs=2,
                                   name=f"trp{h}{jb}{chn}")
                    nc.tensor.transpose(trp[:, 0:65],
                                        yaug16[:, 128 * chn:128 * (chn + 1)],
                                        id16[0:65, 0:65])
                    rec = expp.tile([128, 1], F32, tag=f"rec{e}", bufs=2,
                                    name=f"rec{h}{jb}{chn}")
                    nc.vector.reciprocal(rec[:], trp[:, 64:65])
                    nc.vector.tensor_scalar(
                        y_sb[:, 4 * jb + chn, h, :], trp[:, 0:64],
                        rec[:], None, op0=OP.mult)
            return f

        for jb in range(NQB):
            thunks.append(mk_yaug(jb))
            for fg in range(2 * jb):
                thunks.append(mk_full(jb, fg))
            for dpair in range(2):
                thunks.append(mk_diag(jb, dpair))
            thunks.append(mk_epi(jb))
        return thunks

    def attention_pair(p):
        for a, b in zip_longest(head_thunks(p, 0), head_thunks(p, 1)):
            if a is not None:
                a()
            if b is not None:
                b()

    def send_half(ph):
        a2a_in, a2a_out = a2a[ph]
        for d in range(NCORES):
            nc.sync.dma_start(
                a2a_in[d].rearrange("(p t h dd) -> p t h dd", p=128, t=2, h=2),
                y_sb[:, 2 * d:2 * d + 2, 2 * ph:2 * ph + 2, :])
        nc.gpsimd.collective_compute(
            "AllToAll", OP.bypass, replica_groups=[list(range(NCORES))],
            ins=[a2a_in.opt()], outs=[a2a_out.opt()])

    def recv_half(ph):
        """Unpack half ph into channel-major ycT and fold into accY.

        Source s holds global heads {4*(s%4)+2*ph, +1}, i.e. channels
        [256*(s%4)+128*ph, +128) -> ct = 2*(s%4) + ph, lanes 0..127.
        """
        _, a2a_out = a2a[ph]
        for s in range(NCORES):
            ct = 2 * (s % 4) + ph
            yarr = tail.tile([128, 2, 2, D], F16, tag="yarr", bufs=4,
                             name=f"yarr{ph}{s}")
            nc.sync.dma_start(
                yarr[:],
                a2a_out[s].rearrange("(p t h dd) -> p t h dd", p=128, t=2, h=2))
            try8 = ppc.tile([128, 512], F16, tag="trx", bufs=2,
                            name=f"try{ph}{s}")
            for tl in range(2):
                for hh in range(2):
                    nc.tensor.transpose(
                        try8[64 * hh:64 * (hh + 1), 128 * tl:128 * (tl + 1)],
                        yarr[:, tl, hh], id16[:])
            tt_loc = 2 * (s // 4)
            csl = slice(128 * tt_loc, 128 * (tt_loc + 2))
            eng = (nc.vector, nc.gpsimd)[s % 2]
            eng.tensor_copy(ycT[:, ct, csl], try8[:, 0:256])
            eng.tensor_tensor(accY[:, csl], accY[:, csl], try8[:, 0:256],
                              op=OP.abs_max)

    attention_pair(0)
    send_half(0)
    attention_pair(1)
    recv_half(0)
    send_half(1)
    recv_half(1)

    # ---------------- P5: output quant + projection -------------------------
    psC.close()
    psD = ExitStack()
    ppd = psD.enter_context(tc.tile_pool(name="ppd", bufs=1, space="PSUM"))

    mxyb = tail.tile([128, OT], F32)
    nc.gpsimd.partition_all_reduce(mxyb[:], accY[:], 128, bass_isa.ReduceOp.absmax)
    scyb = mxyb   # in-place
    nc.vector.tensor_scalar(scyb[:], mxyb[:], 1e-5, 1.0 / 127.0,
                            op0=OP.max, op1=OP.mult)
    styb = tail.tile([128, OT], F32)
    nc.vector.reciprocal(styb[:], scyb[:])
    scyb16 = tail.tile([128, OT], F16)
    nc.gpsimd.tensor_copy(scyb16[:], scyb[:])
    osc = tail.tile([128, OTT, 1], F32)
    for tt in range(OTT):
        trso = ppd.tile([128, 128], F16, tag="trso", bufs=1, name=f"trso_{tt}")
        nc.tensor.transpose(trso[:], scyb16[:, 128 * tt:128 * (tt + 1)], id16[:])
        nc.vector.tensor_scalar(osc[:, tt], trso[:, 0:1], swco[:], None,
                                op0=OP.mult)

    ya8 = tail.tile([128, NCT, OT], F8)
    yb8 = tail.tile([128, NCT, OT], F8)
    for ct in range(NCT):
        yq8 = tail.tile([128, OT], I8, tag="yq8", bufs=2, name=f"yq8_{ct}")
        nc.vector.tensor_tensor(yq8[:], ycT[:, ct], styb[:], op=OP.mult)
        nc.scalar.activation(ya8[:, ct], yq8[:], ACTF.Copy)
        nc.gpsimd.tensor_tensor(yb8[:, ct], yq8[:], ya8[:, ct], op=OP.subtract)

    outR = io["out_slice"].rearrange("(tt p) c -> p tt c", p=128)
    for tt in range(OTT):
        pso = ppd.tile([128, C], F32, tag="pso", bufs=2, name=f"pso_{tt}")
        for ctp in range(NCP):
            for si, srcT in enumerate((ya8, yb8)):
                nc.tensor.matmul(
                    pso[:], srcT[:, 2 * ctp:2 * ctp + 2, 128 * tt:128 * (tt + 1)],
                    wo8[:, 2 * ctp:2 * ctp + 2, :],
                    start=(ctp == 0 and si == 0),
                    stop=(ctp == NCP - 1 and si == 1), perf_mode=DR)
        outt = tail.tile([128, C], F32, tag="outt", bufs=2, name=f"outt_{tt}")
        nc.scalar.activation(outt[:], pso[:], ACTF.Copy, scale=osc[:, tt])
        nc.sync.dma_start(outR[:, tt], outt[:])
    psD.close()
    es.close()


def kernel(x, Wq, Wk, Wv, Wo, _trace=False):
    x = np.ascontiguousarray(np.asarray(x, np.float32))
    if "nc" not in _CACHE:
        _CACHE["nc"] = build_program()
    nc = _CACHE["nc"]
    t1, t2 = _host_tables()
    jth = _host_jt()
    ws = {"Wq": np.asarray(Wq, np.float32), "Wk": np.asarray(Wk, np.float32),
          "Wv": np.asarray(Wv, np.float32), "Wo": np.asarray(Wo, np.float32)}
    wstack = np.stack([ws["Wq"], ws["Wk"], ws["Wv"], ws["Wo"]], axis=0)
    woT = np.ascontiguousarray(ws["Wo"].T)
    in_maps = []
    for c in range(NCORES):
        b, j = c // 4, c % 4
        in_maps.append({
            "xT16": np.ascontiguousarray(x[b].T.astype(np.float16)),
            "Wshards": np.ascontiguousarray(
                wstack[:, 128 * c:128 * (c + 1), :].transpose(1, 0, 2)),
            "WqTs": np.ascontiguousarray(ws["Wq"][HD * j:HD * (j + 1), :].T),
            "WkTs": np.ascontiguousarray(ws["Wk"][HD * j:HD * (j + 1), :].T),
            "WvTs": np.ascontiguousarray(ws["Wv"][HD * j:HD * (j + 1), :].T),
            "WoT": woT,
            "ropeT1": t1, "ropeT2": t2, "ropeJT": jth,
        })
    res = run_bass_kernel_spmd(nc, in_maps, list(range(NCORES)), trace=_trace)
    out = np.zeros((B, T, C), np.float32)
    for c in range(NCORES):
        o = np.asarray(res.results[c]["out_slice"])
        out[0, 256 * c:256 * (c + 1)] = o[0:256]
        out[1, 256 * c:256 * (c + 1)] = o[256:512]
    if _trace:
        return out, res
    return out
